# revision 11
# baseline (speedup 1.0000x reference)
"""Trainium2 Bass kernel for the CIN-style layer:

    z   = einsum('btf,byf->bfty', x_0, x_k)            # pairwise outer products
    z   = z.reshape(bs, ts0, f, tsk)                   # flat reinterpretation
    out = einsum('btiy,nty->bni', z, conv_w) + conv_b  # strided conv reduction

Shapes: x_0 (32, 64, 256), x_k (32, 64, 256), conv_w (128, 64, 64),
conv_b (128,) -> out (32, 128, 256).

Math: with i = a*64 + m  (a = i//64, m = i%64) and feature f = 4t + a the
reference reduces to a two-step factorization:

    W2[b,n,t,a]      = sum_y x_k[b,y,4t+a] * conv_w[n,t,y]         (contract y)
    out[b,n,a*64+m]  = sum_t x_0[b,m,4t+a] * W2[b,n,t,a] + conv_b  (contract t)

Sharding: pure data parallel over batch, 4 samples per core, conv_w/conv_b
replicated (no collectives).

Device mapping (v12, all-bf16 compute, fp32 PSUM/output).  Per core,
c = 4*b_loc + a in [0,16), c = 2*c2 + h, t = 2p + q' with pair p in [0,32):

  step 1 (n-stationary): 32 matmuls, one per t-pair p:
      lhsT = WT[:, 128p:+128]      [K=128 (q,y), M=128 (n)]  (bf16)
      rhs  = XKpad[:, 32p:+32]     [K=128 (q,y), N=32 (q',c)] (bf16, zero-
             padded block-diagonal in q==q', padded on-chip from dense)
      -> PSUM [n; 32p+16q'+2c2+h] = W2[c, n, t=2p+q']
  scatter copy (DVE+GpSimd, one per (u-half, h)): PSUM -> SBUF bf16 in the
      shuffle layout  W2n[n; 128c2 + 64h + t].
  shuffle: 8 plain matmuls against a shipped bf16 identity:
      lhsT = W2n[:, 128c2:+128], rhs = I  ->  PSUM [(64h+t); n]; cast
      copies -> w2r bf16.
  step 2: per c-pair c2, PSUM pre-loaded with the bias via a K=1 matmul
      (ones-row (x) bias-row, runs in the DMA-wait shadow), then
      lhsT = X0pad[:, 128c2:+128]  [K=128 (64h+t), M=128 (h',m)]  (bf16,
             block-diagonal in h==h', padded on-chip from dense)
      rhs  = w2r[:, 128c2:+128]    accumulated on top (start=False)
      -> PSUM [64h+m, n] = out + bias, DMA'd to DRAM directly from PSUM.

DMA: sync ring carries xk (dense) + 4 weight chunks (these pace step 1) +
the second output half; scalar ring carries x0-dense+identity, the bias
row, and the first output half.
"""

import numpy as np
import ml_dtypes

BS, TS, F, NF = 32, 64, 256, 128
NCORES = 8
B = BS // NCORES  # 4 local batches per core

F32 = np.float32
BF16 = ml_dtypes.bfloat16


# ---------------------------------------------------------------------------
# Host-side packing
# ---------------------------------------------------------------------------

def _pack_wt(conv_w: np.ndarray) -> np.ndarray:
    # WT[64q+y, 128p+n] = conv_w[n, 2p+q, y]
    wt = conv_w.transpose(1, 2, 0).reshape(32, 2, 64, NF)  # [p, q, y, n]
    wt = wt.transpose(1, 2, 0, 3)                          # [q, y, p, n]
    return np.ascontiguousarray(
        wt.reshape(128, 32 * NF).astype(BF16))


def _pack_xk_dense(xk_shard: np.ndarray) -> np.ndarray:
    # XKD[64q+y, 16p+c] = xk[b, y, 8p+4q+a]   (c = 4b+a)
    xq = xk_shard.reshape(B, TS, 32, 2, 4)       # [b, y, p, q, a]
    src = xq.transpose(3, 1, 2, 0, 4)            # [q, y, p, b, a]
    return np.ascontiguousarray(src.reshape(128, 512).astype(BF16))


def _pack_x0_dense(x0_shard: np.ndarray) -> np.ndarray:
    # X0D[64h+t, 64c2+m] = x0[b(2c2+h), m, 4t+a(2c2+h)]
    xt = x0_shard.reshape(B, TS, TS, 4).transpose(0, 3, 2, 1)  # [b, a, t, m]
    flat = xt.reshape(16, TS, TS)                              # [c, t, m]
    arr = np.empty((2, TS, 8, TS), dtype=F32)                  # [h, t, c2, m]
    for h in (0, 1):
        arr[h] = flat[2 * np.arange(8) + h].transpose(1, 0, 2)
    return np.ascontiguousarray(arr.reshape(128, 512).astype(BF16))


def _unpack_out(out_pack: np.ndarray, out_full: np.ndarray, r: int) -> None:
    # out_pack[64h+m, 128c2+n] = out[4r+b(c), n, a(c)*64+m], c = 2*c2 + h
    o = out_pack.reshape(2, TS, 8, NF)  # [h, m, c2, n]
    for c2 in range(8):
        for h in (0, 1):
            c = 2 * c2 + h
            b, a = divmod(c, 4)
            out_full[4 * r + b, :, a * TS:(a + 1) * TS] = o[h, :, c2, :].T


# ---------------------------------------------------------------------------
# Device program
# ---------------------------------------------------------------------------

_prog_cache = {}


def _emit_body(nc, tc, pool, ps_pool, f32, bf16, xk_d, wt_d, in1_d, bias_d,
               out_d, n_warm=2):
    # PE warm-up on a zeroed bf16 tile while the input DMAs stream in.
    warm_s = pool.tile([128, 512], bf16, tag="warm")
    nc.gpsimd.memset(warm_s[:], 0.0)
    ps_w = ps_pool.tile([128, 512], f32, tag="warm_ps")
    for _ in range(n_warm):
        nc.tensor.matmul(ps_w[:, :], warm_s[:, 0:128], warm_s[:, :],
                         start=True, stop=True)

    # ---- input DMAs ----
    # sync ring: ONLY the 4 weight chunks (these pace step 1; nothing may
    # queue ahead of them).  scalar ring: xk, x0+identity, bias row.
    xkd_s = pool.tile([128, 512], bf16, tag="xkd")
    nc.scalar.dma_start(xkd_s[:], xk_d.ap())
    wt_s = []
    for chunk in range(4):
        t_ = pool.tile([128, 1024], bf16, tag=f"wt{chunk}")
        nc.sync.dma_start(t_[:], wt_d.ap()[:, 1024 * chunk:1024 * (chunk + 1)])
        wt_s.append(t_)
    # scalar ring: x0 dense + identity, then the bias row
    in1_s = pool.tile([128, 640], bf16, tag="in1")
    nc.scalar.dma_start(in1_s[:], in1_d.ap())
    bias_s = pool.tile([1, 512], bf16, tag="bias")
    nc.scalar.dma_start(bias_s[:], bias_d.ap())

    x0d_s = in1_s[:, 0:512]
    ident = in1_s[:, 512:640]

    ones_s = pool.tile([128, 128], bf16, tag="ones")
    nc.gpsimd.memset(ones_s[:], 1.0)

    # zero-pad dense xk into the q-block-diagonal layout on-chip
    xk_pad = pool.tile([128, 1024], bf16, tag="xkpad")
    nc.gpsimd.memset(xk_pad[:], 0.0)
    for q in range(2):
        dst = xk_pad[64 * q:64 * (q + 1), :].rearrange(
            "p (a b) -> p a b", b=32)[:, :, 16 * q:16 * (q + 1)]
        src = xkd_s[64 * q:64 * (q + 1), :].rearrange(
            "p (a b) -> p a b", b=16)
        nc.vector.tensor_copy(dst, src)

    # zero-pad dense x0 into the h-block-diagonal layout on-chip
    x0_pad = pool.tile([128, 1024], bf16, tag="x0pad")
    nc.gpsimd.memset(x0_pad[:], 0.0)
    for h in (0, 1):
        dst = x0_pad[64 * h:64 * (h + 1), :].rearrange(
            "p (c2 hp m) -> p c2 hp m", c2=8, hp=2)[:, :, h, :]
        src = x0d_s[64 * h:64 * (h + 1), :].rearrange(
            "p (c2 m) -> p c2 m", c2=8)
        nc.gpsimd.tensor_copy(dst, src)

    def wt_cols(p):  # lhsT tile [128, 128] for pair p
        chunk, off = divmod(128 * p, 1024)
        return wt_s[chunk][:, off:off + 128]

    # pre-allocate all PSUM tiles (4 tags x bufs=2 = all 8 banks, no reuse)
    ps1, ps_t, ps2 = [], [], []
    for i in range(2):
        ps1_i = ps_pool.tile([128, 512], f32, tag="s1")
        ps_t_i = ps_pool.tile([128, 512], f32, tag="t2")
        ps2_i = ps_pool.tile([128, 512], f32, tag="s2")
        ps1.append(ps1_i)
        ps_t.append(ps_t_i)
        ps2.append(ps2_i)

    # ---- step 1: W2 = wt_p.T @ xk_pad_p (contract (q,y)) ----
    # psum cols 32p+16q'+2c2+h; the scatter copy (split by h across
    # DVE/GpSimd) lands W2n[n; 128c2 + 64h + (2p+q')] = [n; 128c2+64h+t].
    w2n_s = pool.tile([128, 1024], bf16, tag="w2n")

    def emit_s1(u):
        for g in range(2):  # one group per 8-pair weight chunk
            for p in range(16 * u + 8 * g, 16 * u + 8 * g + 8):
                nc.tensor.matmul(
                    ps1[u][:, 32 * (p % 16):32 * (p % 16 + 1)],
                    wt_cols(p),
                    xk_pad[:, 32 * p:32 * (p + 1)],
                    start=True, stop=True,
                )
            tl = slice(16 * g, 16 * (g + 1))
            for h in (0, 1):
                src = ps1[u][:, :].rearrange(
                    "z (tl c2 h) -> z c2 tl h", tl=32, c2=8)[:, :, tl, h]
                dst = w2n_s[:].rearrange(
                    "z (c2 hh uu tl) -> z c2 hh uu tl",
                    c2=8, hh=2, uu=2, tl=32)[:, :, h, u, tl]
                if h == 0:
                    nc.vector.tensor_copy(dst, src)
                else:
                    nc.scalar.copy(dst, src)

    emit_s1(0)

    # bias pre-load of the step-2 PSUM banks: psum[:, (f,n)] = bias[n]
    # (K=1 matmul, ones-row (x) bias-row; runs in the wt-DMA-wait shadow)
    for u in range(2):
        nc.tensor.matmul(ps2[u][:, :], ones_s[0:1, 0:128], bias_s[0:1, :],
                         start=True, stop=False)

    emit_s1(1)

    # ---- shuffle: w2r[64h+t; 128c2+n] via 8 identity matmuls ----
    w2r_s = pool.tile([128, 1024], bf16, tag="w2r")
    for v in range(2):
        for c2 in range(4 * v, 4 * v + 4):
            nc.tensor.matmul(
                ps_t[v][:, 128 * (c2 % 4):128 * (c2 % 4 + 1)],
                w2n_s[:, 128 * c2:128 * (c2 + 1)],
                ident,
                start=True, stop=True,
            )
        for half in (0, 1):
            cols = slice(512 * v + 256 * half, 512 * v + 256 * (half + 1))
            pcols = slice(256 * half, 256 * (half + 1))
            if half == 0:
                nc.vector.tensor_copy(w2r_s[:, cols], ps_t[v][:, pcols])
            else:
                nc.scalar.copy(w2r_s[:, cols], ps_t[v][:, pcols])

    # ---- step 2: psum(bias) += x0l.T @ w2r (contract (64h+t)) ----
    out_s = pool.tile([128, 1024], f32, tag="out")
    for u in range(2):
        for c2 in range(4 * u, 4 * u + 4):
            nc.tensor.matmul(
                ps2[u][:, 128 * (c2 % 4):128 * (c2 % 4 + 1)],
                x0_pad[:, 128 * c2:128 * (c2 + 1)],
                w2r_s[:, 128 * c2:128 * (c2 + 1)],
                start=False, stop=True,
            )
        for half in (0, 1):
            cols = slice(512 * u + 256 * half, 512 * u + 256 * (half + 1))
            pcols = slice(256 * half, 256 * (half + 1))
            if half == 0:
                nc.vector.tensor_copy(out_s[:, cols], ps2[u][:, pcols])
            else:
                nc.scalar.copy(out_s[:, cols], ps2[u][:, pcols])
        eng = nc.scalar if u == 0 else nc.sync
        eng.dma_start(out_d.ap()[:, 512 * u:512 * (u + 1)],
                      out_s[:, 512 * u:512 * (u + 1)])


def _build_program(version=12):
    if version in _prog_cache:
        return _prog_cache[version]

    from contextlib import ExitStack

    import concourse.bacc as bacc
    import concourse.mybir as mybir
    import concourse.tile as tile

    f32 = mybir.dt.float32
    bf16 = mybir.dt.bfloat16
    nc = bacc.Bacc("TRN2", target_bir_lowering=False, debug=False)

    xk_d = nc.dram_tensor("xk_pack", [128, 512], bf16, kind="ExternalInput")
    wt_d = nc.dram_tensor("wt_pack", [128, 4096], bf16, kind="ExternalInput")
    in1_d = nc.dram_tensor("in1_pack", [128, 640], bf16, kind="ExternalInput")
    bias_d = nc.dram_tensor("bias_pack", [1, 512], bf16, kind="ExternalInput")
    out_d = nc.dram_tensor("out_pack", [128, 1024], f32, kind="ExternalOutput")

    with tile.TileContext(nc) as tc, ExitStack() as ctx:
        pool = ctx.enter_context(tc.tile_pool(name="io", bufs=1))
        ps_pool = ctx.enter_context(tc.tile_pool(name="ps", bufs=2, space="PSUM"))
        _emit_body(nc, tc, pool, ps_pool, f32, bf16, xk_d, wt_d, in1_d,
                   bias_d, out_d)

    nc.compile()
    _prog_cache[version] = nc
    return nc


def pack_core_inputs(x_0, x_k, conv_w, conv_b, version=12):
    """Returns in_maps (list of 8 dicts) for run_bass_kernel_spmd."""
    wt = _pack_wt(np.asarray(conv_w, dtype=F32))
    bias4 = np.ascontiguousarray(
        np.tile(np.asarray(conv_b, dtype=F32), 4)[None, :].astype(BF16))
    ident = np.eye(128, dtype=BF16)
    x0 = np.asarray(x_0, dtype=F32)
    xk = np.asarray(x_k, dtype=F32)
    in_maps = []
    for r in range(NCORES):
        in1 = np.concatenate(
            [_pack_x0_dense(x0[B * r:B * (r + 1)]), ident], axis=1)
        in_maps.append({
            "xk_pack": _pack_xk_dense(xk[B * r:B * (r + 1)]),
            "wt_pack": wt,
            "in1_pack": np.ascontiguousarray(in1),
            "bias_pack": bias4,
        })
    return in_maps


VERSION = 12


def kernel(x_0, x_k, conv_w, conv_b):
    from concourse.bass_utils import run_bass_kernel_spmd

    nc = _build_program(VERSION)
    in_maps = pack_core_inputs(x_0, x_k, conv_w, conv_b, version=VERSION)
    res = run_bass_kernel_spmd(nc, in_maps, core_ids=list(range(NCORES)))
    out = np.empty((BS, NF, F), dtype=F32)
    for r in range(NCORES):
        _unpack_out(res.results[r]["out_pack"], out, r)
    return out


# ---------------------------------------------------------------------------
# numpy model of the packed device program (for testing the packing logic)
# ---------------------------------------------------------------------------

def _numpy_model(x_0, x_k, conv_w, conv_b):
    out = np.empty((BS, NF, F), dtype=F32)
    in_maps = pack_core_inputs(x_0, x_k, conv_w, conv_b)
    for r in range(NCORES):
        m = in_maps[r]
        xkd = m["xk_pack"].astype(F32)
        wt = m["wt_pack"].astype(F32)
        x0d = m["in1_pack"][:, :512].astype(F32)
        bias4 = m["bias_pack"].astype(F32)  # [1, 512] = bias tiled 4x
        # on-chip xk padding (q block-diagonal)
        xk_pad = np.zeros((128, 1024), dtype=F32)
        for q in range(2):
            blk = xk_pad[64 * q:64 * (q + 1)].reshape(64, 32, 32)
            blk[:, :, 16 * q:16 * (q + 1)] = (
                xkd[64 * q:64 * (q + 1)].reshape(64, 32, 16))
        # on-chip x0 padding (h block-diagonal)
        x0l = np.zeros((128, 1024), dtype=F32)
        for h in (0, 1):
            blk = x0l[64 * h:64 * (h + 1)].reshape(64, 8, 2, 64)
            blk[:, :, h, :] = x0d[64 * h:64 * (h + 1)].reshape(64, 8, 64)
        # step 1 + scatter copy: W2n[n; 128c2 + 64h + t], t = 2p + q'
        w2n = np.zeros((128, 8, 2, 64), dtype=F32)  # [n, c2, h, t]
        for p in range(32):
            blk = (wt[:, 128 * p:128 * (p + 1)].T
                   @ xk_pad[:, 32 * p:32 * (p + 1)])  # [n, (q',c2,h)]
            blk = blk.reshape(128, 2, 8, 2)
            for qp in range(2):
                w2n[:, :, :, 2 * p + qp] = blk[:, qp].transpose(0, 1, 2)
        w2n = w2n.reshape(128, 1024).astype(BF16).astype(F32)
        # shuffle
        w2r = np.zeros((128, 1024), dtype=F32)
        for c2 in range(8):
            w2r[:, 128 * c2:128 * (c2 + 1)] = (
                w2n[:, 128 * c2:128 * (c2 + 1)].T)
        w2r = w2r.astype(BF16).astype(F32)
        # step 2 (psum pre-loaded with bias via ones (x) bias4)
        out_pack = np.empty((128, 1024), dtype=F32)
        for u in range(2):
            out_pack[:, 512 * u:512 * (u + 1)] = bias4
        for c2 in range(8):
            out_pack[:, 128 * c2:128 * (c2 + 1)] += (
                x0l[:, 128 * c2:128 * (c2 + 1)].T
                @ w2r[:, 128 * c2:128 * (c2 + 1)]
            )
        _unpack_out(out_pack, out, r)
    return out


# revision 12
# speedup vs baseline: 1.1143x; 1.1143x over previous
"""Trainium2 Bass kernel for the CIN-style layer:

    z   = einsum('btf,byf->bfty', x_0, x_k)            # pairwise outer products
    z   = z.reshape(bs, ts0, f, tsk)                   # flat reinterpretation
    out = einsum('btiy,nty->bni', z, conv_w) + conv_b  # strided conv reduction

Shapes: x_0 (32, 64, 256), x_k (32, 64, 256), conv_w (128, 64, 64),
conv_b (128,) -> out (32, 128, 256).

Math: with i = a*64 + m  (a = i//64, m = i%64) and feature f = 4t + a the
reference reduces to a two-step factorization:

    W2[b,n,t,a]      = sum_y x_k[b,y,4t+a] * conv_w[n,t,y]         (contract y)
    out[b,n,a*64+m]  = sum_t x_0[b,m,4t+a] * W2[b,n,t,a] + conv_b  (contract t)

Sharding: pure data parallel over batch, 4 samples per core, conv_w/conv_b
replicated (no collectives).

Device mapping (v12, all-bf16 compute, fp32 PSUM/output).  Per core,
c = 4*b_loc + a in [0,16), c = 2*c2 + h, t = 2p + q' with pair p in [0,32):

  step 1 (n-stationary): 32 matmuls, one per t-pair p:
      lhsT = WT[:, 128p:+128]      [K=128 (q,y), M=128 (n)]  (bf16)
      rhs  = XKpad[:, 32p:+32]     [K=128 (q,y), N=32 (q',c)] (bf16, zero-
             padded block-diagonal in q==q', padded on-chip from dense)
      -> PSUM [n; 32p+16q'+2c2+h] = W2[c, n, t=2p+q']
  scatter copy (DVE+GpSimd, one per (u-half, h)): PSUM -> SBUF bf16 in the
      shuffle layout  W2n[n; 128c2 + 64h + t].
  shuffle: 8 plain matmuls against a shipped bf16 identity:
      lhsT = W2n[:, 128c2:+128], rhs = I  ->  PSUM [(64h+t); n]; cast
      copies -> w2r bf16.
  step 2: per c-pair c2, PSUM pre-loaded with the bias via a K=1 matmul
      (ones-row (x) bias-row, runs in the DMA-wait shadow), then
      lhsT = X0pad[:, 128c2:+128]  [K=128 (64h+t), M=128 (h',m)]  (bf16,
             block-diagonal in h==h', padded on-chip from dense)
      rhs  = w2r[:, 128c2:+128]    accumulated on top (start=False)
      -> PSUM [64h+m, n] = out + bias, DMA'd to DRAM directly from PSUM.

DMA: sync ring carries xk (dense) + 4 weight chunks (these pace step 1) +
the second output half; scalar ring carries x0-dense+identity, the bias
row, and the first output half.
"""

import numpy as np
import ml_dtypes

BS, TS, F, NF = 32, 64, 256, 128
NCORES = 8
B = BS // NCORES  # 4 local batches per core

F32 = np.float32
BF16 = ml_dtypes.bfloat16


# ---------------------------------------------------------------------------
# Host-side packing
# ---------------------------------------------------------------------------

def _pack_wt(conv_w: np.ndarray) -> np.ndarray:
    # WT[64q+y, 128p+n] = conv_w[n, 2p+q, y]
    wt = conv_w.transpose(1, 2, 0).reshape(32, 2, 64, NF)  # [p, q, y, n]
    wt = wt.transpose(1, 2, 0, 3)                          # [q, y, p, n]
    return np.ascontiguousarray(
        wt.reshape(128, 32 * NF).astype(BF16))


def _pack_xk_dense(xk_shard: np.ndarray) -> np.ndarray:
    # XKD[64q+y, 16p+c] = xk[b, y, 8p+4q+a]   (c = 4b+a)
    xq = xk_shard.reshape(B, TS, 32, 2, 4)       # [b, y, p, q, a]
    src = xq.transpose(3, 1, 2, 0, 4)            # [q, y, p, b, a]
    return np.ascontiguousarray(src.reshape(128, 512).astype(BF16))


def _pack_x0_dense(x0_shard: np.ndarray) -> np.ndarray:
    # X0D[64h+t, 64c2+m] = x0[b(2c2+h), m, 4t+a(2c2+h)]
    xt = x0_shard.reshape(B, TS, TS, 4).transpose(0, 3, 2, 1)  # [b, a, t, m]
    flat = xt.reshape(16, TS, TS)                              # [c, t, m]
    arr = np.empty((2, TS, 8, TS), dtype=F32)                  # [h, t, c2, m]
    for h in (0, 1):
        arr[h] = flat[2 * np.arange(8) + h].transpose(1, 0, 2)
    return np.ascontiguousarray(arr.reshape(128, 512).astype(BF16))


def _unpack_out(out_pack: np.ndarray, out_full: np.ndarray, r: int) -> None:
    # out_pack[64h+m, 128c2+n] = out[4r+b(c), n, a(c)*64+m], c = 2*c2 + h
    o = out_pack.reshape(2, TS, 8, NF)  # [h, m, c2, n]
    for c2 in range(8):
        for h in (0, 1):
            c = 2 * c2 + h
            b, a = divmod(c, 4)
            out_full[4 * r + b, :, a * TS:(a + 1) * TS] = o[h, :, c2, :].T


# ---------------------------------------------------------------------------
# Device program
# ---------------------------------------------------------------------------

_prog_cache = {}


def _emit_body(nc, tc, pool, ps_pool, f32, bf16, xk_d, wt_d, in1_d, bias_d,
               out_d, n_warm=2):
    # PE warm-up on a zeroed bf16 tile while the input DMAs stream in.
    warm_s = pool.tile([128, 512], bf16, tag="warm")
    nc.gpsimd.memset(warm_s[:], 0.0)
    ps_w = ps_pool.tile([128, 512], f32, tag="warm_ps")
    for _ in range(n_warm):
        nc.tensor.matmul(ps_w[:, :], warm_s[:, 0:128], warm_s[:, :],
                         start=True, stop=True)

    # ---- input DMAs ----
    # sync ring: xk+identity first (small, gates step-1's rhs and the
    # shuffle), then the 4 weight chunks (these pace step 1).  The
    # scalar/ACT ring is starved behind the sync ring, so it only gets
    # latency-tolerant loads: the bias row (needed mid-kernel, tiny) and
    # x0-dense (needed only by step 2).
    xkd_s = pool.tile([128, 640], bf16, tag="xkd")
    nc.sync.dma_start(xkd_s[:], xk_d.ap())
    wt_s = []
    for chunk in range(4):
        t_ = pool.tile([128, 1024], bf16, tag=f"wt{chunk}")
        nc.sync.dma_start(t_[:], wt_d.ap()[:, 1024 * chunk:1024 * (chunk + 1)])
        wt_s.append(t_)
    bias_s = pool.tile([1, 512], bf16, tag="bias")
    nc.scalar.dma_start(bias_s[:], bias_d.ap())
    in1_s = pool.tile([128, 512], bf16, tag="in1")
    nc.scalar.dma_start(in1_s[:], in1_d.ap())

    x0d_s = in1_s[:, 0:512]
    ident = xkd_s[:, 512:640]

    ones_s = pool.tile([128, 128], bf16, tag="ones")
    nc.gpsimd.memset(ones_s[:], 1.0)

    # zero-pad dense xk into the q-block-diagonal layout on-chip
    xk_pad = pool.tile([128, 1024], bf16, tag="xkpad")
    nc.gpsimd.memset(xk_pad[:], 0.0)
    for q in range(2):
        dst = xk_pad[64 * q:64 * (q + 1), :].rearrange(
            "p (a b) -> p a b", b=32)[:, :, 16 * q:16 * (q + 1)]
        src = xkd_s[64 * q:64 * (q + 1), 0:512].rearrange(
            "p (a b) -> p a b", b=16)
        nc.vector.tensor_copy(dst, src)

    # zero-pad dense x0 into the h-block-diagonal layout on-chip
    x0_pad = pool.tile([128, 1024], bf16, tag="x0pad")
    nc.gpsimd.memset(x0_pad[:], 0.0)
    for h in (0, 1):
        dst = x0_pad[64 * h:64 * (h + 1), :].rearrange(
            "p (c2 hp m) -> p c2 hp m", c2=8, hp=2)[:, :, h, :]
        src = x0d_s[64 * h:64 * (h + 1), :].rearrange(
            "p (c2 m) -> p c2 m", c2=8)
        nc.gpsimd.tensor_copy(dst, src)

    def wt_cols(p):  # lhsT tile [128, 128] for pair p
        chunk, off = divmod(128 * p, 1024)
        return wt_s[chunk][:, off:off + 128]

    # pre-allocate all PSUM tiles (4 tags x bufs=2 = all 8 banks, no reuse)
    ps1, ps_t, ps2 = [], [], []
    for i in range(2):
        ps1_i = ps_pool.tile([128, 512], f32, tag="s1")
        ps_t_i = ps_pool.tile([128, 512], f32, tag="t2")
        ps2_i = ps_pool.tile([128, 512], f32, tag="s2")
        ps1.append(ps1_i)
        ps_t.append(ps_t_i)
        ps2.append(ps2_i)

    # ---- step 1: W2 = wt_p.T @ xk_pad_p (contract (q,y)) ----
    # psum cols 32p+16q'+2c2+h; the scatter copy (split by h across
    # DVE/GpSimd) lands W2n[n; 128c2 + 64h + (2p+q')] = [n; 128c2+64h+t].
    w2n_s = pool.tile([128, 1024], bf16, tag="w2n")

    def emit_s1(u):
        for g in range(2):  # one group per 8-pair weight chunk
            for p in range(16 * u + 8 * g, 16 * u + 8 * g + 8):
                nc.tensor.matmul(
                    ps1[u][:, 32 * (p % 16):32 * (p % 16 + 1)],
                    wt_cols(p),
                    xk_pad[:, 32 * p:32 * (p + 1)],
                    start=True, stop=True,
                )
            tl = slice(16 * g, 16 * (g + 1))
            for h in (0, 1):
                src = ps1[u][:, :].rearrange(
                    "z (tl c2 h) -> z c2 tl h", tl=32, c2=8)[:, :, tl, h]
                dst = w2n_s[:].rearrange(
                    "z (c2 hh uu tl) -> z c2 hh uu tl",
                    c2=8, hh=2, uu=2, tl=32)[:, :, h, u, tl]
                if h == 0:
                    nc.vector.tensor_copy(dst, src)
                else:
                    nc.scalar.copy(dst, src)

    emit_s1(0)

    # bias pre-load of the step-2 PSUM banks: psum[:, (f,n)] = bias[n]
    # (K=1 matmul, ones-row (x) bias-row; runs in the wt-DMA-wait shadow)
    for u in range(2):
        nc.tensor.matmul(ps2[u][:, :], ones_s[0:1, 0:128], bias_s[0:1, :],
                         start=True, stop=False)

    emit_s1(1)

    # ---- shuffle: w2r[64h+t; 128c2+n] via 8 identity matmuls ----
    w2r_s = pool.tile([128, 1024], bf16, tag="w2r")
    for v in range(2):
        for c2 in range(4 * v, 4 * v + 4):
            nc.tensor.matmul(
                ps_t[v][:, 128 * (c2 % 4):128 * (c2 % 4 + 1)],
                w2n_s[:, 128 * c2:128 * (c2 + 1)],
                ident,
                start=True, stop=True,
            )
        for half in (0, 1):
            cols = slice(512 * v + 256 * half, 512 * v + 256 * (half + 1))
            pcols = slice(256 * half, 256 * (half + 1))
            if half == 0:
                nc.vector.tensor_copy(w2r_s[:, cols], ps_t[v][:, pcols])
            else:
                nc.scalar.copy(w2r_s[:, cols], ps_t[v][:, pcols])

    # ---- step 2: psum(bias) += x0l.T @ w2r (contract (64h+t)) ----
    out_s = pool.tile([128, 1024], f32, tag="out")
    for u in range(2):
        for c2 in range(4 * u, 4 * u + 4):
            nc.tensor.matmul(
                ps2[u][:, 128 * (c2 % 4):128 * (c2 % 4 + 1)],
                x0_pad[:, 128 * c2:128 * (c2 + 1)],
                w2r_s[:, 128 * c2:128 * (c2 + 1)],
                start=False, stop=True,
            )
        for half in (0, 1):
            cols = slice(512 * u + 256 * half, 512 * u + 256 * (half + 1))
            pcols = slice(256 * half, 256 * (half + 1))
            if half == 0:
                nc.vector.tensor_copy(out_s[:, cols], ps2[u][:, pcols])
            else:
                nc.scalar.copy(out_s[:, cols], ps2[u][:, pcols])
        eng = nc.scalar if u == 0 else nc.sync
        eng.dma_start(out_d.ap()[:, 512 * u:512 * (u + 1)],
                      out_s[:, 512 * u:512 * (u + 1)])


def _build_program(version=12):
    if version in _prog_cache:
        return _prog_cache[version]

    from contextlib import ExitStack

    import concourse.bacc as bacc
    import concourse.mybir as mybir
    import concourse.tile as tile

    f32 = mybir.dt.float32
    bf16 = mybir.dt.bfloat16
    nc = bacc.Bacc("TRN2", target_bir_lowering=False, debug=False)

    xk_d = nc.dram_tensor("xk_pack", [128, 640], bf16, kind="ExternalInput")
    wt_d = nc.dram_tensor("wt_pack", [128, 4096], bf16, kind="ExternalInput")
    in1_d = nc.dram_tensor("in1_pack", [128, 512], bf16, kind="ExternalInput")
    bias_d = nc.dram_tensor("bias_pack", [1, 512], bf16, kind="ExternalInput")
    out_d = nc.dram_tensor("out_pack", [128, 1024], f32, kind="ExternalOutput")

    with tile.TileContext(nc) as tc, ExitStack() as ctx:
        pool = ctx.enter_context(tc.tile_pool(name="io", bufs=1))
        ps_pool = ctx.enter_context(tc.tile_pool(name="ps", bufs=2, space="PSUM"))
        _emit_body(nc, tc, pool, ps_pool, f32, bf16, xk_d, wt_d, in1_d,
                   bias_d, out_d)

    nc.compile()
    _prog_cache[version] = nc
    return nc


def pack_core_inputs(x_0, x_k, conv_w, conv_b, version=12):
    """Returns in_maps (list of 8 dicts) for run_bass_kernel_spmd."""
    wt = _pack_wt(np.asarray(conv_w, dtype=F32))
    bias4 = np.ascontiguousarray(
        np.tile(np.asarray(conv_b, dtype=F32), 4)[None, :].astype(BF16))
    ident = np.eye(128, dtype=BF16)
    x0 = np.asarray(x_0, dtype=F32)
    xk = np.asarray(x_k, dtype=F32)
    in_maps = []
    for r in range(NCORES):
        xkp = np.concatenate(
            [_pack_xk_dense(xk[B * r:B * (r + 1)]), ident], axis=1)
        in_maps.append({
            "xk_pack": np.ascontiguousarray(xkp),
            "wt_pack": wt,
            "in1_pack": _pack_x0_dense(x0[B * r:B * (r + 1)]),
            "bias_pack": bias4,
        })
    return in_maps


VERSION = 12


def kernel(x_0, x_k, conv_w, conv_b):
    from concourse.bass_utils import run_bass_kernel_spmd

    nc = _build_program(VERSION)
    in_maps = pack_core_inputs(x_0, x_k, conv_w, conv_b, version=VERSION)
    res = run_bass_kernel_spmd(nc, in_maps, core_ids=list(range(NCORES)))
    out = np.empty((BS, NF, F), dtype=F32)
    for r in range(NCORES):
        _unpack_out(res.results[r]["out_pack"], out, r)
    return out


# ---------------------------------------------------------------------------
# numpy model of the packed device program (for testing the packing logic)
# ---------------------------------------------------------------------------

def _numpy_model(x_0, x_k, conv_w, conv_b):
    out = np.empty((BS, NF, F), dtype=F32)
    in_maps = pack_core_inputs(x_0, x_k, conv_w, conv_b)
    for r in range(NCORES):
        m = in_maps[r]
        xkd = m["xk_pack"][:, :512].astype(F32)
        wt = m["wt_pack"].astype(F32)
        x0d = m["in1_pack"].astype(F32)
        bias4 = m["bias_pack"].astype(F32)  # [1, 512] = bias tiled 4x
        # on-chip xk padding (q block-diagonal)
        xk_pad = np.zeros((128, 1024), dtype=F32)
        for q in range(2):
            blk = xk_pad[64 * q:64 * (q + 1)].reshape(64, 32, 32)
            blk[:, :, 16 * q:16 * (q + 1)] = (
                xkd[64 * q:64 * (q + 1)].reshape(64, 32, 16))
        # on-chip x0 padding (h block-diagonal)
        x0l = np.zeros((128, 1024), dtype=F32)
        for h in (0, 1):
            blk = x0l[64 * h:64 * (h + 1)].reshape(64, 8, 2, 64)
            blk[:, :, h, :] = x0d[64 * h:64 * (h + 1)].reshape(64, 8, 64)
        # step 1 + scatter copy: W2n[n; 128c2 + 64h + t], t = 2p + q'
        w2n = np.zeros((128, 8, 2, 64), dtype=F32)  # [n, c2, h, t]
        for p in range(32):
            blk = (wt[:, 128 * p:128 * (p + 1)].T
                   @ xk_pad[:, 32 * p:32 * (p + 1)])  # [n, (q',c2,h)]
            blk = blk.reshape(128, 2, 8, 2)
            for qp in range(2):
                w2n[:, :, :, 2 * p + qp] = blk[:, qp].transpose(0, 1, 2)
        w2n = w2n.reshape(128, 1024).astype(BF16).astype(F32)
        # shuffle
        w2r = np.zeros((128, 1024), dtype=F32)
        for c2 in range(8):
            w2r[:, 128 * c2:128 * (c2 + 1)] = (
                w2n[:, 128 * c2:128 * (c2 + 1)].T)
        w2r = w2r.astype(BF16).astype(F32)
        # step 2 (psum pre-loaded with bias via ones (x) bias4)
        out_pack = np.empty((128, 1024), dtype=F32)
        for u in range(2):
            out_pack[:, 512 * u:512 * (u + 1)] = bias4
        for c2 in range(8):
            out_pack[:, 128 * c2:128 * (c2 + 1)] += (
                x0l[:, 128 * c2:128 * (c2 + 1)].T
                @ w2r[:, 128 * c2:128 * (c2 + 1)]
            )
        _unpack_out(out_pack, out, r)
    return out


# revision 13
# speedup vs baseline: 1.1201x; 1.0052x over previous
"""Trainium2 Bass kernel for the CIN-style layer:

    z   = einsum('btf,byf->bfty', x_0, x_k)            # pairwise outer products
    z   = z.reshape(bs, ts0, f, tsk)                   # flat reinterpretation
    out = einsum('btiy,nty->bni', z, conv_w) + conv_b  # strided conv reduction

Shapes: x_0 (32, 64, 256), x_k (32, 64, 256), conv_w (128, 64, 64),
conv_b (128,) -> out (32, 128, 256).

Math: with i = a*64 + m  (a = i//64, m = i%64) and feature f = 4t + a the
reference reduces to a two-step factorization:

    W2[b,n,t,a]      = sum_y x_k[b,y,4t+a] * conv_w[n,t,y]         (contract y)
    out[b,n,a*64+m]  = sum_t x_0[b,m,4t+a] * W2[b,n,t,a] + conv_b  (contract t)

Sharding: pure data parallel over batch, 4 samples per core, conv_w/conv_b
replicated (no collectives).

Device mapping (v12, all-bf16 compute, fp32 PSUM/output).  Per core,
c = 4*b_loc + a in [0,16), c = 2*c2 + h, t = 2p + q' with pair p in [0,32):

  step 1 (n-stationary): 32 matmuls, one per t-pair p:
      lhsT = WT[:, 128p:+128]      [K=128 (q,y), M=128 (n)]  (bf16)
      rhs  = XKpad[:, 32p:+32]     [K=128 (q,y), N=32 (q',c)] (bf16, zero-
             padded block-diagonal in q==q', padded on-chip from dense)
      -> PSUM [n; 32p+16q'+2c2+h] = W2[c, n, t=2p+q']
  scatter copy (DVE+GpSimd, one per (u-half, h)): PSUM -> SBUF bf16 in the
      shuffle layout  W2n[n; 128c2 + 64h + t].
  shuffle: 8 plain matmuls against a shipped bf16 identity:
      lhsT = W2n[:, 128c2:+128], rhs = I  ->  PSUM [(64h+t); n]; cast
      copies -> w2r bf16.
  step 2: per c-pair c2, PSUM pre-loaded with the bias via a K=1 matmul
      (ones-row (x) bias-row, runs in the DMA-wait shadow), then
      lhsT = X0pad[:, 128c2:+128]  [K=128 (64h+t), M=128 (h',m)]  (bf16,
             block-diagonal in h==h', padded on-chip from dense)
      rhs  = w2r[:, 128c2:+128]    accumulated on top (start=False)
      -> PSUM [64h+m, n] = out + bias, DMA'd to DRAM directly from PSUM.

DMA: sync ring carries xk (dense) + 4 weight chunks (these pace step 1) +
the second output half; scalar ring carries x0-dense+identity, the bias
row, and the first output half.
"""

import numpy as np
import ml_dtypes

BS, TS, F, NF = 32, 64, 256, 128
NCORES = 8
B = BS // NCORES  # 4 local batches per core

F32 = np.float32
BF16 = ml_dtypes.bfloat16


# ---------------------------------------------------------------------------
# Host-side packing
# ---------------------------------------------------------------------------

def _pack_wt(conv_w: np.ndarray) -> np.ndarray:
    # WT[64q+y, 128p+n] = conv_w[n, 2p+q, y]
    wt = conv_w.transpose(1, 2, 0).reshape(32, 2, 64, NF)  # [p, q, y, n]
    wt = wt.transpose(1, 2, 0, 3)                          # [q, y, p, n]
    return np.ascontiguousarray(
        wt.reshape(128, 32 * NF).astype(BF16))


def _pack_xk_dense(xk_shard: np.ndarray) -> np.ndarray:
    # XKD[64q+y, 16p+c] = xk[b, y, 8p+4q+a]   (c = 4b+a)
    xq = xk_shard.reshape(B, TS, 32, 2, 4)       # [b, y, p, q, a]
    src = xq.transpose(3, 1, 2, 0, 4)            # [q, y, p, b, a]
    return np.ascontiguousarray(src.reshape(128, 512).astype(BF16))


def _pack_x0_dense(x0_shard: np.ndarray) -> np.ndarray:
    # X0D[64h+t, 64c2+m] = x0[b(2c2+h), m, 4t+a(2c2+h)]
    xt = x0_shard.reshape(B, TS, TS, 4).transpose(0, 3, 2, 1)  # [b, a, t, m]
    flat = xt.reshape(16, TS, TS)                              # [c, t, m]
    arr = np.empty((2, TS, 8, TS), dtype=F32)                  # [h, t, c2, m]
    for h in (0, 1):
        arr[h] = flat[2 * np.arange(8) + h].transpose(1, 0, 2)
    return np.ascontiguousarray(arr.reshape(128, 512).astype(BF16))


def _unpack_out(out_pack: np.ndarray, out_full: np.ndarray, r: int) -> None:
    # out_pack[64h+m, 128c2+n] = out[4r+b(c), n, a(c)*64+m], c = 2*c2 + h
    o = out_pack.reshape(2, TS, 8, NF)  # [h, m, c2, n]
    for c2 in range(8):
        for h in (0, 1):
            c = 2 * c2 + h
            b, a = divmod(c, 4)
            out_full[4 * r + b, :, a * TS:(a + 1) * TS] = o[h, :, c2, :].T


# ---------------------------------------------------------------------------
# Device program
# ---------------------------------------------------------------------------

_prog_cache = {}


def _emit_body(nc, tc, pool, ps_pool, f32, bf16, xk_d, wt_d, in1_d, bias_d,
               out_d, n_warm=2):
    # PE warm-up on a zeroed bf16 tile while the input DMAs stream in.
    warm_s = pool.tile([128, 512], bf16, tag="warm")
    nc.gpsimd.memset(warm_s[:], 0.0)
    ps_w = ps_pool.tile([128, 512], f32, tag="warm_ps")
    for _ in range(n_warm):
        nc.tensor.matmul(ps_w[:, :], warm_s[:, 0:128], warm_s[:, :],
                         start=True, stop=True)

    # ---- input DMAs ----
    # sync ring: xk+identity first (small, gates step-1's rhs and the
    # shuffle), then the 4 weight chunks (these pace step 1).  The
    # scalar/ACT ring is starved behind the sync ring, so it only gets
    # latency-tolerant loads: the bias row (needed mid-kernel, tiny) and
    # x0-dense (needed only by step 2).
    xkd_s = pool.tile([128, 640], bf16, tag="xkd")
    nc.sync.dma_start(xkd_s[:], xk_d.ap())
    wt_s = []
    for chunk in range(4):
        t_ = pool.tile([128, 1024], bf16, tag=f"wt{chunk}")
        nc.sync.dma_start(t_[:], wt_d.ap()[:, 1024 * chunk:1024 * (chunk + 1)])
        wt_s.append(t_)
    bias_s = pool.tile([1, 512], bf16, tag="bias")
    nc.scalar.dma_start(bias_s[:], bias_d.ap())
    in1_s = pool.tile([128, 512], bf16, tag="in1")
    nc.scalar.dma_start(in1_s[:], in1_d.ap())

    x0d_s = in1_s[:, 0:512]
    ident = xkd_s[:, 512:640]

    ones_s = pool.tile([128, 128], bf16, tag="ones")
    nc.gpsimd.memset(ones_s[:], 1.0)

    # zero-pad dense xk into the q-block-diagonal layout on-chip
    xk_pad = pool.tile([128, 1024], bf16, tag="xkpad")
    nc.gpsimd.memset(xk_pad[:], 0.0)
    for q in range(2):
        dst = xk_pad[64 * q:64 * (q + 1), :].rearrange(
            "p (a b) -> p a b", b=32)[:, :, 16 * q:16 * (q + 1)]
        src = xkd_s[64 * q:64 * (q + 1), 0:512].rearrange(
            "p (a b) -> p a b", b=16)
        nc.vector.tensor_copy(dst, src)

    # zero-pad dense x0 into the h-block-diagonal layout on-chip
    x0_pad = pool.tile([128, 1024], bf16, tag="x0pad")
    nc.gpsimd.memset(x0_pad[:], 0.0)
    for h in (0, 1):
        dst = x0_pad[64 * h:64 * (h + 1), :].rearrange(
            "p (c2 hp m) -> p c2 hp m", c2=8, hp=2)[:, :, h, :]
        src = x0d_s[64 * h:64 * (h + 1), :].rearrange(
            "p (c2 m) -> p c2 m", c2=8)
        nc.gpsimd.tensor_copy(dst, src)

    def wt_cols(p):  # lhsT tile [128, 128] for pair p
        chunk, off = divmod(128 * p, 1024)
        return wt_s[chunk][:, off:off + 128]

    # pre-allocate all PSUM tiles (4 tags x bufs=2 = all 8 banks, no reuse)
    ps1, ps_t, ps2 = [], [], []
    for i in range(2):
        ps1_i = ps_pool.tile([128, 512], f32, tag="s1")
        ps_t_i = ps_pool.tile([128, 512], f32, tag="t2")
        ps2_i = ps_pool.tile([128, 512], f32, tag="s2")
        ps1.append(ps1_i)
        ps_t.append(ps_t_i)
        ps2.append(ps2_i)

    # ---- step 1: W2 = wt_p.T @ xk_pad_p (contract (q,y)) ----
    # psum cols 32p+16q'+2c2+h; the scatter copy (split by h across
    # DVE/GpSimd) lands W2n[n; 128c2 + 64h + (2p+q')] = [n; 128c2+64h+t].
    w2n_s = pool.tile([128, 1024], bf16, tag="w2n")

    def emit_s1(u):
        for p in range(16 * u, 16 * u + 16):
            nc.tensor.matmul(
                ps1[u][:, 32 * (p % 16):32 * (p % 16 + 1)],
                wt_cols(p),
                xk_pad[:, 32 * p:32 * (p + 1)],
                start=True, stop=True,
            )
        for h in (0, 1):
            src = ps1[u][:, :].rearrange(
                "z (tl c2 h) -> z c2 tl h", tl=32, c2=8)[:, :, :, h]
            dst = w2n_s[:].rearrange(
                "z (c2 hh uu tl) -> z c2 hh uu tl",
                c2=8, hh=2, uu=2, tl=32)[:, :, h, u, :]
            if h == 0:
                nc.vector.tensor_copy(dst, src)
            else:
                nc.scalar.copy(dst, src)

    emit_s1(0)

    # bias pre-load of the step-2 PSUM banks: psum[:, (f,n)] = bias[n]
    # (K=1 matmul, ones-row (x) bias-row; runs in the wt-DMA-wait shadow)
    for u in range(2):
        nc.tensor.matmul(ps2[u][:, :], ones_s[0:1, 0:128], bias_s[0:1, :],
                         start=True, stop=False)

    emit_s1(1)

    # ---- shuffle: w2r[64h+t; 128c2+n] via 8 identity matmuls ----
    w2r_s = pool.tile([128, 1024], bf16, tag="w2r")
    for v in range(2):
        for c2 in range(4 * v, 4 * v + 4):
            nc.tensor.matmul(
                ps_t[v][:, 128 * (c2 % 4):128 * (c2 % 4 + 1)],
                w2n_s[:, 128 * c2:128 * (c2 + 1)],
                ident,
                start=True, stop=True,
            )
        for half in (0, 1):
            cols = slice(512 * v + 256 * half, 512 * v + 256 * (half + 1))
            pcols = slice(256 * half, 256 * (half + 1))
            if half == 0:
                nc.vector.tensor_copy(w2r_s[:, cols], ps_t[v][:, pcols])
            else:
                nc.scalar.copy(w2r_s[:, cols], ps_t[v][:, pcols])

    # ---- step 2: psum(bias) += x0l.T @ w2r (contract (64h+t)) ----
    out_s = pool.tile([128, 1024], f32, tag="out")
    for u in range(2):
        for c2 in range(4 * u, 4 * u + 4):
            nc.tensor.matmul(
                ps2[u][:, 128 * (c2 % 4):128 * (c2 % 4 + 1)],
                x0_pad[:, 128 * c2:128 * (c2 + 1)],
                w2r_s[:, 128 * c2:128 * (c2 + 1)],
                start=False, stop=True,
            )
        for half in (0, 1):
            cols = slice(512 * u + 256 * half, 512 * u + 256 * (half + 1))
            pcols = slice(256 * half, 256 * (half + 1))
            if half == 0:
                nc.vector.tensor_copy(out_s[:, cols], ps2[u][:, pcols])
            else:
                nc.scalar.copy(out_s[:, cols], ps2[u][:, pcols])
        eng = nc.scalar if u == 0 else nc.sync
        eng.dma_start(out_d.ap()[:, 512 * u:512 * (u + 1)],
                      out_s[:, 512 * u:512 * (u + 1)])


def _build_program(version=12):
    if version in _prog_cache:
        return _prog_cache[version]

    from contextlib import ExitStack

    import concourse.bacc as bacc
    import concourse.mybir as mybir
    import concourse.tile as tile

    f32 = mybir.dt.float32
    bf16 = mybir.dt.bfloat16
    nc = bacc.Bacc("TRN2", target_bir_lowering=False, debug=False)

    xk_d = nc.dram_tensor("xk_pack", [128, 640], bf16, kind="ExternalInput")
    wt_d = nc.dram_tensor("wt_pack", [128, 4096], bf16, kind="ExternalInput")
    in1_d = nc.dram_tensor("in1_pack", [128, 512], bf16, kind="ExternalInput")
    bias_d = nc.dram_tensor("bias_pack", [1, 512], bf16, kind="ExternalInput")
    out_d = nc.dram_tensor("out_pack", [128, 1024], f32, kind="ExternalOutput")

    with tile.TileContext(nc) as tc, ExitStack() as ctx:
        pool = ctx.enter_context(tc.tile_pool(name="io", bufs=1))
        ps_pool = ctx.enter_context(tc.tile_pool(name="ps", bufs=2, space="PSUM"))
        _emit_body(nc, tc, pool, ps_pool, f32, bf16, xk_d, wt_d, in1_d,
                   bias_d, out_d)

    nc.compile()
    _prog_cache[version] = nc
    return nc


def pack_core_inputs(x_0, x_k, conv_w, conv_b, version=12):
    """Returns in_maps (list of 8 dicts) for run_bass_kernel_spmd."""
    wt = _pack_wt(np.asarray(conv_w, dtype=F32))
    bias4 = np.ascontiguousarray(
        np.tile(np.asarray(conv_b, dtype=F32), 4)[None, :].astype(BF16))
    ident = np.eye(128, dtype=BF16)
    x0 = np.asarray(x_0, dtype=F32)
    xk = np.asarray(x_k, dtype=F32)
    in_maps = []
    for r in range(NCORES):
        xkp = np.concatenate(
            [_pack_xk_dense(xk[B * r:B * (r + 1)]), ident], axis=1)
        in_maps.append({
            "xk_pack": np.ascontiguousarray(xkp),
            "wt_pack": wt,
            "in1_pack": _pack_x0_dense(x0[B * r:B * (r + 1)]),
            "bias_pack": bias4,
        })
    return in_maps


VERSION = 12


def kernel(x_0, x_k, conv_w, conv_b):
    from concourse.bass_utils import run_bass_kernel_spmd

    nc = _build_program(VERSION)
    in_maps = pack_core_inputs(x_0, x_k, conv_w, conv_b, version=VERSION)
    res = run_bass_kernel_spmd(nc, in_maps, core_ids=list(range(NCORES)))
    out = np.empty((BS, NF, F), dtype=F32)
    for r in range(NCORES):
        _unpack_out(res.results[r]["out_pack"], out, r)
    return out


# ---------------------------------------------------------------------------
# numpy model of the packed device program (for testing the packing logic)
# ---------------------------------------------------------------------------

def _numpy_model(x_0, x_k, conv_w, conv_b):
    out = np.empty((BS, NF, F), dtype=F32)
    in_maps = pack_core_inputs(x_0, x_k, conv_w, conv_b)
    for r in range(NCORES):
        m = in_maps[r]
        xkd = m["xk_pack"][:, :512].astype(F32)
        wt = m["wt_pack"].astype(F32)
        x0d = m["in1_pack"].astype(F32)
        bias4 = m["bias_pack"].astype(F32)  # [1, 512] = bias tiled 4x
        # on-chip xk padding (q block-diagonal)
        xk_pad = np.zeros((128, 1024), dtype=F32)
        for q in range(2):
            blk = xk_pad[64 * q:64 * (q + 1)].reshape(64, 32, 32)
            blk[:, :, 16 * q:16 * (q + 1)] = (
                xkd[64 * q:64 * (q + 1)].reshape(64, 32, 16))
        # on-chip x0 padding (h block-diagonal)
        x0l = np.zeros((128, 1024), dtype=F32)
        for h in (0, 1):
            blk = x0l[64 * h:64 * (h + 1)].reshape(64, 8, 2, 64)
            blk[:, :, h, :] = x0d[64 * h:64 * (h + 1)].reshape(64, 8, 64)
        # step 1 + scatter copy: W2n[n; 128c2 + 64h + t], t = 2p + q'
        w2n = np.zeros((128, 8, 2, 64), dtype=F32)  # [n, c2, h, t]
        for p in range(32):
            blk = (wt[:, 128 * p:128 * (p + 1)].T
                   @ xk_pad[:, 32 * p:32 * (p + 1)])  # [n, (q',c2,h)]
            blk = blk.reshape(128, 2, 8, 2)
            for qp in range(2):
                w2n[:, :, :, 2 * p + qp] = blk[:, qp].transpose(0, 1, 2)
        w2n = w2n.reshape(128, 1024).astype(BF16).astype(F32)
        # shuffle
        w2r = np.zeros((128, 1024), dtype=F32)
        for c2 in range(8):
            w2r[:, 128 * c2:128 * (c2 + 1)] = (
                w2n[:, 128 * c2:128 * (c2 + 1)].T)
        w2r = w2r.astype(BF16).astype(F32)
        # step 2 (psum pre-loaded with bias via ones (x) bias4)
        out_pack = np.empty((128, 1024), dtype=F32)
        for u in range(2):
            out_pack[:, 512 * u:512 * (u + 1)] = bias4
        for c2 in range(8):
            out_pack[:, 128 * c2:128 * (c2 + 1)] += (
                x0l[:, 128 * c2:128 * (c2 + 1)].T
                @ w2r[:, 128 * c2:128 * (c2 + 1)]
            )
        _unpack_out(out_pack, out, r)
    return out


# revision 14
# speedup vs baseline: 1.1465x; 1.0235x over previous
"""Trainium2 Bass kernel for the CIN-style layer:

    z   = einsum('btf,byf->bfty', x_0, x_k)            # pairwise outer products
    z   = z.reshape(bs, ts0, f, tsk)                   # flat reinterpretation
    out = einsum('btiy,nty->bni', z, conv_w) + conv_b  # strided conv reduction

Shapes: x_0 (32, 64, 256), x_k (32, 64, 256), conv_w (128, 64, 64),
conv_b (128,) -> out (32, 128, 256).

Math: with i = a*64 + m  (a = i//64, m = i%64) and feature f = 4t + a the
reference reduces to a two-step factorization:

    W2[b,n,t,a]      = sum_y x_k[b,y,4t+a] * conv_w[n,t,y]         (contract y)
    out[b,n,a*64+m]  = sum_t x_0[b,m,4t+a] * W2[b,n,t,a] + conv_b  (contract t)

Sharding: pure data parallel over batch, 4 samples per core, conv_w/conv_b
replicated (no collectives).

Device mapping (v12, all-bf16 compute, fp32 PSUM/output).  Per core,
c = 4*b_loc + a in [0,16), c = 2*c2 + h, t = 2p + q' with pair p in [0,32):

  step 1 (n-stationary): 32 matmuls, one per t-pair p:
      lhsT = WT[:, 128p:+128]      [K=128 (q,y), M=128 (n)]  (bf16)
      rhs  = XKpad[:, 32p:+32]     [K=128 (q,y), N=32 (q',c)] (bf16, zero-
             padded block-diagonal in q==q', padded on-chip from dense)
      -> PSUM [n; 32p+16q'+2c2+h] = W2[c, n, t=2p+q']
  scatter copy (DVE+GpSimd, one per (u-half, h)): PSUM -> SBUF bf16 in the
      shuffle layout  W2n[n; 128c2 + 64h + t].
  shuffle: 8 plain matmuls against a shipped bf16 identity:
      lhsT = W2n[:, 128c2:+128], rhs = I  ->  PSUM [(64h+t); n]; cast
      copies -> w2r bf16.
  step 2: per c-pair c2, PSUM pre-loaded with the bias via a K=1 matmul
      (ones-row (x) bias-row, runs in the DMA-wait shadow), then
      lhsT = X0pad[:, 128c2:+128]  [K=128 (64h+t), M=128 (h',m)]  (bf16,
             block-diagonal in h==h', padded on-chip from dense)
      rhs  = w2r[:, 128c2:+128]    accumulated on top (start=False)
      -> PSUM [64h+m, n] = out + bias, DMA'd to DRAM directly from PSUM.

DMA: sync ring carries xk (dense) + 4 weight chunks (these pace step 1) +
the second output half; scalar ring carries x0-dense+identity, the bias
row, and the first output half.
"""

import numpy as np
import ml_dtypes

BS, TS, F, NF = 32, 64, 256, 128
NCORES = 8
B = BS // NCORES  # 4 local batches per core

F32 = np.float32
BF16 = ml_dtypes.bfloat16


# ---------------------------------------------------------------------------
# Host-side packing
# ---------------------------------------------------------------------------

def _pack_wt(conv_w: np.ndarray) -> np.ndarray:
    # WT[64q+y, 128p+n] = conv_w[n, 2p+q, y]
    wt = conv_w.transpose(1, 2, 0).reshape(32, 2, 64, NF)  # [p, q, y, n]
    wt = wt.transpose(1, 2, 0, 3)                          # [q, y, p, n]
    return np.ascontiguousarray(
        wt.reshape(128, 32 * NF).astype(BF16))


def _pack_xk_dense(xk_shard: np.ndarray) -> np.ndarray:
    # XKD[64q+y, 16p+c] = xk[b, y, 8p+4q+a]   (c = 4b+a)
    xq = xk_shard.reshape(B, TS, 32, 2, 4)       # [b, y, p, q, a]
    src = xq.transpose(3, 1, 2, 0, 4)            # [q, y, p, b, a]
    return np.ascontiguousarray(src.reshape(128, 512).astype(BF16))


def _pack_x0_dense(x0_shard: np.ndarray) -> np.ndarray:
    # X0D[64h+t, 64c2+m] = x0[b(2c2+h), m, 4t+a(2c2+h)]
    xt = x0_shard.reshape(B, TS, TS, 4).transpose(0, 3, 2, 1)  # [b, a, t, m]
    flat = xt.reshape(16, TS, TS)                              # [c, t, m]
    arr = np.empty((2, TS, 8, TS), dtype=F32)                  # [h, t, c2, m]
    for h in (0, 1):
        arr[h] = flat[2 * np.arange(8) + h].transpose(1, 0, 2)
    return np.ascontiguousarray(arr.reshape(128, 512).astype(BF16))


def _unpack_out(out_pack: np.ndarray, out_full: np.ndarray, r: int) -> None:
    # out_pack[64h+m, 128c2+n] = out[4r+b(c), n, a(c)*64+m], c = 2*c2 + h
    o = out_pack.reshape(2, TS, 8, NF)  # [h, m, c2, n]
    for c2 in range(8):
        for h in (0, 1):
            c = 2 * c2 + h
            b, a = divmod(c, 4)
            out_full[4 * r + b, :, a * TS:(a + 1) * TS] = o[h, :, c2, :].T


# ---------------------------------------------------------------------------
# Device program
# ---------------------------------------------------------------------------

_prog_cache = {}


def _emit_body(nc, tc, pool, ps_pool, f32, bf16, xk_d, wt_d, in1_d, bias_d,
               out_d, n_warm=2):
    # PE warm-up on a zeroed bf16 tile while the input DMAs stream in.
    warm_s = pool.tile([128, 512], bf16, tag="warm")
    nc.gpsimd.memset(warm_s[:], 0.0)
    ps_w = ps_pool.tile([128, 512], f32, tag="warm_ps")
    for _ in range(n_warm):
        nc.tensor.matmul(ps_w[:, :], warm_s[:, 0:128], warm_s[:, :],
                         start=True, stop=True)

    # ---- input DMAs ----
    # sync ring: xk+identity first (small, gates step-1's rhs and the
    # shuffle), then the 4 weight chunks (these pace step 1).  The
    # scalar/ACT ring is starved behind the sync ring, so it only gets
    # latency-tolerant loads: the bias row (needed mid-kernel, tiny) and
    # x0-dense (needed only by step 2).
    xkd_s = pool.tile([128, 640], bf16, tag="xkd")
    nc.sync.dma_start(xkd_s[:], xk_d.ap())
    wt_s = []
    for chunk in range(4):
        t_ = pool.tile([128, 1024], bf16, tag=f"wt{chunk}")
        nc.sync.dma_start(t_[:], wt_d.ap()[:, 1024 * chunk:1024 * (chunk + 1)])
        wt_s.append(t_)
    bias_s = pool.tile([1, 512], bf16, tag="bias")
    nc.scalar.dma_start(bias_s[:], bias_d.ap())
    in1_s = pool.tile([128, 512], bf16, tag="in1")
    nc.scalar.dma_start(in1_s[:], in1_d.ap())

    x0d_s = in1_s[:, 0:512]
    ident = xkd_s[:, 512:640]

    ones_s = pool.tile([128, 128], bf16, tag="ones")
    nc.gpsimd.memset(ones_s[:], 1.0)

    # zero-pad dense xk into the q-block-diagonal layout on-chip
    xk_pad = pool.tile([128, 1024], bf16, tag="xkpad")
    nc.gpsimd.memset(xk_pad[:], 0.0)
    for q in range(2):
        dst = xk_pad[64 * q:64 * (q + 1), :].rearrange(
            "p (a b) -> p a b", b=32)[:, :, 16 * q:16 * (q + 1)]
        src = xkd_s[64 * q:64 * (q + 1), 0:512].rearrange(
            "p (a b) -> p a b", b=16)
        nc.vector.tensor_copy(dst, src)

    # zero-pad dense x0 into the h-block-diagonal layout on-chip
    x0_pad = pool.tile([128, 1024], bf16, tag="x0pad")
    nc.gpsimd.memset(x0_pad[:], 0.0)
    for h in (0, 1):
        dst = x0_pad[64 * h:64 * (h + 1), :].rearrange(
            "p (c2 hp m) -> p c2 hp m", c2=8, hp=2)[:, :, h, :]
        src = x0d_s[64 * h:64 * (h + 1), :].rearrange(
            "p (c2 m) -> p c2 m", c2=8)
        nc.gpsimd.tensor_copy(dst, src)

    def wt_cols(p):  # lhsT tile [128, 128] for pair p
        chunk, off = divmod(128 * p, 1024)
        return wt_s[chunk][:, off:off + 128]

    # pre-allocate all PSUM tiles (4 tags x bufs=2 = all 8 banks, no reuse)
    ps1, ps_t, ps2 = [], [], []
    for i in range(2):
        ps1_i = ps_pool.tile([128, 512], f32, tag="s1")
        ps_t_i = ps_pool.tile([128, 512], f32, tag="t2")
        ps2_i = ps_pool.tile([128, 512], f32, tag="s2")
        ps1.append(ps1_i)
        ps_t.append(ps_t_i)
        ps2.append(ps2_i)

    # ---- step 1: W2 = wt_p.T @ xk_pad_p (contract (q,y)) ----
    # psum cols 32p+16q'+2c2+h; the scatter copy (split by h across
    # DVE/GpSimd) lands W2n[n; 128c2 + 64h + (2p+q')] = [n; 128c2+64h+t].
    w2n_s = pool.tile([128, 1024], bf16, tag="w2n")

    def emit_s1(u):
        for p in range(16 * u, 16 * u + 16):
            nc.tensor.matmul(
                ps1[u][:, 32 * (p % 16):32 * (p % 16 + 1)],
                wt_cols(p),
                xk_pad[:, 32 * p:32 * (p + 1)],
                start=True, stop=True,
            )
        for h in (0, 1):
            src = ps1[u][:, :].rearrange(
                "z (tl c2 h) -> z c2 tl h", tl=32, c2=8)[:, :, :, h]
            dst = w2n_s[:].rearrange(
                "z (c2 hh uu tl) -> z c2 hh uu tl",
                c2=8, hh=2, uu=2, tl=32)[:, :, h, u, :]
            if h == 0 or u == 1:
                nc.vector.tensor_copy(dst, src)
            else:
                nc.scalar.copy(dst, src)

    emit_s1(0)

    # bias pre-load of the step-2 PSUM banks: psum[:, (f,n)] = bias[n]
    # (K=1 matmul, ones-row (x) bias-row; runs in the wt-DMA-wait shadow)
    for u in range(2):
        nc.tensor.matmul(ps2[u][:, :], ones_s[0:1, 0:128], bias_s[0:1, :],
                         start=True, stop=False)

    emit_s1(1)

    # ---- shuffle: w2r[64h+t; 128c2+n] via 8 identity matmuls ----
    w2r_s = pool.tile([128, 1024], bf16, tag="w2r")
    for v in range(2):
        for c2 in range(4 * v, 4 * v + 4):
            nc.tensor.matmul(
                ps_t[v][:, 128 * (c2 % 4):128 * (c2 % 4 + 1)],
                w2n_s[:, 128 * c2:128 * (c2 + 1)],
                ident,
                start=True, stop=True,
            )
        for half in (0, 1):
            cols = slice(512 * v + 256 * half, 512 * v + 256 * (half + 1))
            pcols = slice(256 * half, 256 * (half + 1))
            if half == 0:
                nc.vector.tensor_copy(w2r_s[:, cols], ps_t[v][:, pcols])
            else:
                nc.scalar.copy(w2r_s[:, cols], ps_t[v][:, pcols])

    # ---- step 2: psum(bias) += x0l.T @ w2r (contract (64h+t)) ----
    out_s = pool.tile([128, 1024], f32, tag="out")
    for u in range(2):
        for c2 in range(4 * u, 4 * u + 4):
            nc.tensor.matmul(
                ps2[u][:, 128 * (c2 % 4):128 * (c2 % 4 + 1)],
                x0_pad[:, 128 * c2:128 * (c2 + 1)],
                w2r_s[:, 128 * c2:128 * (c2 + 1)],
                start=False, stop=True,
            )
        for half in (0, 1):
            cols = slice(512 * u + 256 * half, 512 * u + 256 * (half + 1))
            pcols = slice(256 * half, 256 * (half + 1))
            if half == 0:
                nc.vector.tensor_copy(out_s[:, cols], ps2[u][:, pcols])
            else:
                nc.scalar.copy(out_s[:, cols], ps2[u][:, pcols])
        eng = nc.scalar if u == 0 else nc.sync
        eng.dma_start(out_d.ap()[:, 512 * u:512 * (u + 1)],
                      out_s[:, 512 * u:512 * (u + 1)])


def _build_program(version=12):
    if version in _prog_cache:
        return _prog_cache[version]

    from contextlib import ExitStack

    import concourse.bacc as bacc
    import concourse.mybir as mybir
    import concourse.tile as tile

    f32 = mybir.dt.float32
    bf16 = mybir.dt.bfloat16
    nc = bacc.Bacc("TRN2", target_bir_lowering=False, debug=False)

    xk_d = nc.dram_tensor("xk_pack", [128, 640], bf16, kind="ExternalInput")
    wt_d = nc.dram_tensor("wt_pack", [128, 4096], bf16, kind="ExternalInput")
    in1_d = nc.dram_tensor("in1_pack", [128, 512], bf16, kind="ExternalInput")
    bias_d = nc.dram_tensor("bias_pack", [1, 512], bf16, kind="ExternalInput")
    out_d = nc.dram_tensor("out_pack", [128, 1024], f32, kind="ExternalOutput")

    with tile.TileContext(nc) as tc, ExitStack() as ctx:
        pool = ctx.enter_context(tc.tile_pool(name="io", bufs=1))
        ps_pool = ctx.enter_context(tc.tile_pool(name="ps", bufs=2, space="PSUM"))
        _emit_body(nc, tc, pool, ps_pool, f32, bf16, xk_d, wt_d, in1_d,
                   bias_d, out_d)

    nc.compile()
    _prog_cache[version] = nc
    return nc


def pack_core_inputs(x_0, x_k, conv_w, conv_b, version=12):
    """Returns in_maps (list of 8 dicts) for run_bass_kernel_spmd."""
    wt = _pack_wt(np.asarray(conv_w, dtype=F32))
    bias4 = np.ascontiguousarray(
        np.tile(np.asarray(conv_b, dtype=F32), 4)[None, :].astype(BF16))
    ident = np.eye(128, dtype=BF16)
    x0 = np.asarray(x_0, dtype=F32)
    xk = np.asarray(x_k, dtype=F32)
    in_maps = []
    for r in range(NCORES):
        xkp = np.concatenate(
            [_pack_xk_dense(xk[B * r:B * (r + 1)]), ident], axis=1)
        in_maps.append({
            "xk_pack": np.ascontiguousarray(xkp),
            "wt_pack": wt,
            "in1_pack": _pack_x0_dense(x0[B * r:B * (r + 1)]),
            "bias_pack": bias4,
        })
    return in_maps


VERSION = 12


def kernel(x_0, x_k, conv_w, conv_b):
    from concourse.bass_utils import run_bass_kernel_spmd

    nc = _build_program(VERSION)
    in_maps = pack_core_inputs(x_0, x_k, conv_w, conv_b, version=VERSION)
    res = run_bass_kernel_spmd(nc, in_maps, core_ids=list(range(NCORES)))
    out = np.empty((BS, NF, F), dtype=F32)
    for r in range(NCORES):
        _unpack_out(res.results[r]["out_pack"], out, r)
    return out


# ---------------------------------------------------------------------------
# numpy model of the packed device program (for testing the packing logic)
# ---------------------------------------------------------------------------

def _numpy_model(x_0, x_k, conv_w, conv_b):
    out = np.empty((BS, NF, F), dtype=F32)
    in_maps = pack_core_inputs(x_0, x_k, conv_w, conv_b)
    for r in range(NCORES):
        m = in_maps[r]
        xkd = m["xk_pack"][:, :512].astype(F32)
        wt = m["wt_pack"].astype(F32)
        x0d = m["in1_pack"].astype(F32)
        bias4 = m["bias_pack"].astype(F32)  # [1, 512] = bias tiled 4x
        # on-chip xk padding (q block-diagonal)
        xk_pad = np.zeros((128, 1024), dtype=F32)
        for q in range(2):
            blk = xk_pad[64 * q:64 * (q + 1)].reshape(64, 32, 32)
            blk[:, :, 16 * q:16 * (q + 1)] = (
                xkd[64 * q:64 * (q + 1)].reshape(64, 32, 16))
        # on-chip x0 padding (h block-diagonal)
        x0l = np.zeros((128, 1024), dtype=F32)
        for h in (0, 1):
            blk = x0l[64 * h:64 * (h + 1)].reshape(64, 8, 2, 64)
            blk[:, :, h, :] = x0d[64 * h:64 * (h + 1)].reshape(64, 8, 64)
        # step 1 + scatter copy: W2n[n; 128c2 + 64h + t], t = 2p + q'
        w2n = np.zeros((128, 8, 2, 64), dtype=F32)  # [n, c2, h, t]
        for p in range(32):
            blk = (wt[:, 128 * p:128 * (p + 1)].T
                   @ xk_pad[:, 32 * p:32 * (p + 1)])  # [n, (q',c2,h)]
            blk = blk.reshape(128, 2, 8, 2)
            for qp in range(2):
                w2n[:, :, :, 2 * p + qp] = blk[:, qp].transpose(0, 1, 2)
        w2n = w2n.reshape(128, 1024).astype(BF16).astype(F32)
        # shuffle
        w2r = np.zeros((128, 1024), dtype=F32)
        for c2 in range(8):
            w2r[:, 128 * c2:128 * (c2 + 1)] = (
                w2n[:, 128 * c2:128 * (c2 + 1)].T)
        w2r = w2r.astype(BF16).astype(F32)
        # step 2 (psum pre-loaded with bias via ones (x) bias4)
        out_pack = np.empty((128, 1024), dtype=F32)
        for u in range(2):
            out_pack[:, 512 * u:512 * (u + 1)] = bias4
        for c2 in range(8):
            out_pack[:, 128 * c2:128 * (c2 + 1)] += (
                x0l[:, 128 * c2:128 * (c2 + 1)].T
                @ w2r[:, 128 * c2:128 * (c2 + 1)]
            )
        _unpack_out(out_pack, out, r)
    return out
